# revision 9
# baseline (speedup 1.0000x reference)
"""Trainium2 Bass kernel for nn_Change_length (ragged sequence resampling).

Reference semantics (per batch b, valid length Lb from a boolean mask):
  - compact valid tokens to the front (stable) -> seq [L, D]
  - padded_out[j] = mean of seq rows [floor(j*Lb/T), ceil((j+1)*Lb/T))   (area pool)
  - fractional-overlap weights W[j,i] = clip(min(i+1, (j+1)s) - max(i, j*s), 0),
    s = Lb/T; mean = (W@seq)/Wsum, msq = (W@seq^2)/Wsum,
    std_out[j] = sqrt(clip(msq - mean^2, EPS))
  - out_mask = ones [B, T]

Both supports for bin j live in K=5 consecutive rows starting at
base_j = floor(j*s) (s <= 4 since Lb <= 4096, T = 1024). Strategy per core
(pure data parallel, one batch element per core):
  - per 128-bin tile, one indirect DMA gathers 5 consecutive rows per bin
    (index = clamped window start, one index per partition)
  - TensorE: diagonal-weight matmuls accumulate pool / mean-num / msq-num in
    PSUM (weights pre-divided by cnt / Wsum, pre-shifted for the clamp)
  - ScalarE/VectorE epilogue: var = msq - mean^2, clip at EPS, sqrt
Host precomputes only O(T*K) index/weight tables from the mask.
"""

import numpy as np

import concourse.bacc as bacc
import concourse.bass as bass
import concourse.mybir as mybir
import concourse.tile as tile
from concourse.alu_op_type import AluOpType
from contextlib import ExitStack

EPS = 1e-12
B = 8
L = 4096
D = 256
T = 1024
K = 5
P = 128
M = T // P            # 8 jtiles of 128 bins
F32 = mybir.dt.float32
F32R = mybir.dt.float32r


# ----------------------------------------------------------------- host tables
def _make_tables(Lb):
    """Gather window starts and folded weights for valid length Lb.

    Returns (sstart [T] int32, wa [T, K] f32, ww [T, K] f32) where slot k of
    bin j corresponds to row sstart[j] + k of the compacted sequence."""
    if Lb == 0:
        return (np.zeros(T, np.int32), np.zeros((T, K), np.float32),
                np.zeros((T, K), np.float32))

    j = np.arange(T, dtype=np.float64)
    Lf = float(Lb)
    s_idx = np.floor(j * Lf / T)
    e_idx = np.ceil((j + 1.0) * Lf / T)
    cnt = np.maximum(e_idx - s_idx, 1.0).astype(np.float32)
    step = Lf / T
    start = j * step
    end = start + step
    base = s_idx.astype(np.int64)
    sstart = np.minimum(base, L - K)            # clamp so rows stay in bounds

    ks = np.arange(K, dtype=np.int64)[None, :]
    i = sstart[:, None] + ks                     # row gathered into slot k
    # pool weights: indicator(s_idx <= i < e_idx) / cnt  (1/cnt in f32 as ref)
    ind = (i >= base[:, None]) & (i < e_idx[:, None].astype(np.int64))
    wa = (np.float32(1.0) / cnt)[:, None] * ind.astype(np.float32)

    w = np.minimum(i + 1.0, end[:, None]) - np.maximum(i.astype(np.float64),
                                                       start[:, None])
    w = np.clip(w, 0.0, None).astype(np.float32)
    wsum = np.maximum(w.sum(axis=1, dtype=np.float32), np.float32(EPS))
    ww = (w / wsum[:, None]).astype(np.float32)
    return sstart.astype(np.int32), wa, ww


def _pack_tables(sstart, wa, ww):
    """Device layouts: idx32 [128, 8] i32 ([p, m] = window start for bin
    j=128m+p) and wa_p/ww_p [128, 40] f32 ([p, m*5+k])."""
    wa_p = wa.reshape(M, P, K).transpose(1, 0, 2).reshape(P, M * K).copy()
    ww_p = ww.reshape(M, P, K).transpose(1, 0, 2).reshape(P, M * K).copy()
    idx32 = np.ascontiguousarray(sstart.reshape(M, P).T, np.int32)
    return idx32, wa_p, ww_p


def _make_ident5():
    ident = np.zeros((P, K, P), np.float32)
    for k in range(K):
        np.fill_diagonal(ident[:, k, :], 1.0)
    return ident


# -------------------------------------------------------------- device program
def build_program(matmul_dtype=F32R):
    nc = bacc.Bacc("TRN2", target_bir_lowering=False, debug=False)

    x_d = nc.dram_tensor("xb", [L, D], F32, kind="ExternalInput")
    idx_d = nc.dram_tensor("gidx", [P, M], mybir.dt.int32,
                           kind="ExternalInput")
    wa_d = nc.dram_tensor("wa", [P, M * K], F32, kind="ExternalInput")
    ww_d = nc.dram_tensor("ww", [P, M * K], F32, kind="ExternalInput")
    id_d = nc.dram_tensor("ident5", [P, K, P], F32, kind="ExternalInput")
    pool_d = nc.dram_tensor("pool_out", [T, D], F32, kind="ExternalOutput")
    std_d = nc.dram_tensor("std_out", [T, D], F32, kind="ExternalOutput")

    mm = matmul_dtype

    with tile.TileContext(nc) as tc, ExitStack() as ctx:
        consts = ctx.enter_context(tc.tile_pool(name="consts", bufs=1))
        gpool = ctx.enter_context(tc.tile_pool(name="gather", bufs=3))
        spool = ctx.enter_context(tc.tile_pool(name="squares", bufs=3))
        dpool = ctx.enter_context(tc.tile_pool(name="diags", bufs=3))
        opool = ctx.enter_context(tc.tile_pool(name="outs", bufs=3))
        ppool = ctx.enter_context(tc.tile_pool(name="psum", bufs=2,
                                               space="PSUM"))

        idx_t = consts.tile([P, M], mybir.dt.int32)
        wa_t = consts.tile([P, M * K], F32)
        ww_t = consts.tile([P, M * K], F32)
        id_t = consts.tile([P, K, P], F32)
        nc.sync.dma_start(out=idx_t[:], in_=idx_d[:])
        nc.sync.dma_start(out=wa_t[:], in_=wa_d[:])
        nc.sync.dma_start(out=ww_t[:], in_=ww_d[:])
        nc.sync.dma_start(out=id_t[:], in_=id_d[:])

        for m in range(M):
            G = gpool.tile([P, K, D], mm, tag="g")
            for k in range(K):
                # one row per partition per instruction (the only indirect
                # shape the DGE lowers correctly); slot k = window row +k
                nc.gpsimd.indirect_dma_start(
                    out=G[:, k, :], out_offset=None, in_=x_d[:].bitcast(mm),
                    in_offset=bass.IndirectOffsetOnAxis(
                        ap=idx_t[:, m:m + 1], axis=0),
                    element_offset=k * D,
                )
            gsq = spool.tile([P, K, D], mm, tag="gsq")
            nc.scalar.activation(gsq[:], G[:],
                                 mybir.ActivationFunctionType.Square)

            adiag = dpool.tile([P, K, P], mm, tag="adiag")
            wdiag = dpool.tile([P, K, P], mm, tag="wdiag")
            nc.vector.tensor_tensor(
                adiag[:], id_t[:],
                wa_t[:, m * K:(m + 1) * K].to_broadcast([P, K, P]),
                AluOpType.mult)
            nc.vector.tensor_tensor(
                wdiag[:], id_t[:],
                ww_t[:, m * K:(m + 1) * K].to_broadcast([P, K, P]),
                AluOpType.mult)

            pool_p = ppool.tile([P, D], F32, tag="pp")
            mean_p = ppool.tile([P, D], F32, tag="mp")
            msq_p = ppool.tile([P, D], F32, tag="sp")
            for k in range(K):
                nc.tensor.matmul(pool_p[:], lhsT=adiag[:, k, :],
                                 rhs=G[:, k, :],
                                 start=(k == 0), stop=(k == K - 1))
            for k in range(K):
                nc.tensor.matmul(mean_p[:], lhsT=wdiag[:, k, :],
                                 rhs=G[:, k, :],
                                 start=(k == 0), stop=(k == K - 1))
            for k in range(K):
                nc.tensor.matmul(msq_p[:], lhsT=wdiag[:, k, :],
                                 rhs=gsq[:, k, :],
                                 start=(k == 0), stop=(k == K - 1))

            pool_s = opool.tile([P, D], F32, tag="pool_s")
            nc.scalar.copy(pool_s[:], pool_p[:])
            m2 = opool.tile([P, D], F32, tag="m2")
            nc.scalar.activation(m2[:], mean_p[:],
                                 mybir.ActivationFunctionType.Square)
            v = opool.tile([P, D], F32, tag="v")
            nc.vector.tensor_tensor(v[:], msq_p[:], m2[:],
                                    AluOpType.subtract)
            nc.vector.tensor_scalar_max(v[:], v[:], float(EPS))
            std_s = opool.tile([P, D], F32, tag="std_s")
            nc.scalar.activation(std_s[:], v[:],
                                 mybir.ActivationFunctionType.Sqrt)

            nc.sync.dma_start(out=pool_d[m * P:(m + 1) * P, :],
                              in_=pool_s[:])
            nc.sync.dma_start(out=std_d[m * P:(m + 1) * P, :],
                              in_=std_s[:])

    nc.compile()
    return nc


_NC_CACHE = {}


def _get_program(matmul_dtype=None):
    import os
    if matmul_dtype is None:
        matmul_dtype = {"f32": F32, "f32r": F32R}[
            os.environ.get("CL_MM_DTYPE", "f32r")]
    key = str(matmul_dtype)
    if key not in _NC_CACHE:
        _NC_CACHE[key] = build_program(matmul_dtype)
    return _NC_CACHE[key]


def make_in_maps(x, mask):
    """Per-core input dicts (core b <- batch b)."""
    ident = _make_ident5()
    in_maps = []
    for b in range(B):
        mrow = np.asarray(mask[b])
        Lb = int(mrow.sum())
        xb = np.asarray(x[b], np.float32)
        if Lb and not mrow[:Lb].all():
            # non-prefix mask: compact valid tokens to the front (the device
            # program gathers consecutive rows, which assumes compacted input)
            xc = np.zeros_like(xb)
            xc[:Lb] = xb[mrow]
            xb = xc
        sstart, wa, ww = _make_tables(Lb)
        idx32, wa_p, ww_p = _pack_tables(sstart, wa, ww)
        in_maps.append({
            "xb": np.ascontiguousarray(xb),
            "gidx": idx32,
            "wa": wa_p,
            "ww": ww_p,
            "ident5": ident,
        })
    return in_maps


def kernel(x, mask, finallength):
    from concourse.bass_utils import run_bass_kernel_spmd

    x = np.asarray(x)
    mask = np.asarray(mask)
    assert int(finallength) == T
    assert x.shape == (B, L, D)

    nc = _get_program()
    in_maps = make_in_maps(x, mask)
    res = run_bass_kernel_spmd(nc, in_maps, core_ids=list(range(B)))

    pool = np.stack([r["pool_out"] for r in res.results]).astype(np.float32)
    std = np.stack([r["std_out"] for r in res.results]).astype(np.float32)
    out_mask = np.ones((B, T), dtype=bool)
    return pool, out_mask, std
